# revision 6
# baseline (speedup 1.0000x reference)
"""Embedding lookup (char-to-vector) on 8 Trainium2 NeuronCores.

Reference computation: out[t, f, l*64:(l+1)*64] = char2vec[x[t, f, l]]
with x: [256, 256, 8] int ids, char2vec: [8000, 64] f32.

Strategy (data-parallel, per the sharding hint):
  - Shard x along the first (timestep) dim: 32 rows -> 65536 lookups per core.
  - Replicate the 2 MB embedding table to every core (stays in HBM; the
    gather reads it directly).
  - On each core, loop over chunks of Q indices:
      idx DMA-in -> gpsimd.dma_gather (SWDGE gather, 256 B per index)
      -> HWDGE DMA-out of the gathered [128, Q/128, 64] tile.
  - Indices are pre-permuted host-side so that dma_gather's natural SBUF
    layout (index at list position i lands on partition i%128, column
    i//128) dumps to DRAM as the exact row-major [N, 64] output — no
    on-chip or host transpose.

dma_gather operand details (measured/validated on HW):
  - index operand: int16, SBUF [128, Q/16]; logical position i is read from
    partition i%16, column i//16, and the 16-partition block must be
    replicated 8x (one copy per Q7 GPSIMD core).
  - single_packet=False is required for Q > 1024: packet coalescing hits
    the 64-descriptor packet ceiling and wedges the SDMA engine.
  - throughput is descriptor-bound at ~8-9.5 ns per 256 B descriptor,
    independent of chunk size, packet mode, core count, or index locality.
"""

import numpy as np

VOCAB = 8000
EMB = 64
T, F, L = 256, 256, 8
NCORES = 8
N_PER_CORE = (T // NCORES) * F * L  # 65536 lookups per core
Q = 4096                            # indices per dma_gather chunk
NCHUNK = N_PER_CORE // Q
C = Q // 128                        # vectors per partition per chunk

_CACHE = {}


def _build_nc(reps=1, internal_out=False):
    """Per-core program. reps>1 wraps the chunk loop in a hardware loop
    (used only for differential timing); internal_out keeps the big output
    in device DRAM (timing builds only)."""
    import concourse.bacc as bacc
    import concourse.mybir as mybir
    from concourse.tile import TileContext
    from concourse.library_config import mlp

    nc = bacc.Bacc("TRN2", target_bir_lowering=False, debug=False)
    idx = nc.dram_tensor(
        "idx", [NCHUNK, 128, Q // 16], mybir.dt.int16, kind="ExternalInput"
    )
    table = nc.dram_tensor(
        "table", [VOCAB, EMB], mybir.dt.float32, kind="ExternalInput"
    )
    out = nc.dram_tensor(
        "out",
        [N_PER_CORE, EMB],
        mybir.dt.float32,
        kind="Internal" if internal_out else "ExternalOutput",
    )
    chk = None
    if internal_out:
        chk = nc.dram_tensor("chk", [1, 16], mybir.dt.float32, kind="ExternalOutput")
    with TileContext(nc) as tc:
        nc.gpsimd.load_library(mlp)
        with (
            tc.tile_pool(name="idxp", bufs=2) as idxp,
            tc.tile_pool(name="embp", bufs=4) as embp,
        ):
            with tc.For_i(0, reps, 1):
                for k in range(NCHUNK):
                    idx_tile = idxp.tile([128, Q // 16], mybir.dt.int16)
                    nc.sync.dma_start(idx_tile[:, :], idx[k, :, :])
                    emb_tile = embp.tile([128, C * EMB], mybir.dt.float32)
                    emb3 = emb_tile[:, :].rearrange("p (c e) -> p c e", e=EMB)
                    nc.gpsimd.dma_gather(
                        emb3, table[:, :], idx_tile[:, :], Q, Q, EMB,
                        single_packet=False,
                    )
                    out_view = out[k * Q:(k + 1) * Q, :].rearrange(
                        "(p c) e -> p (c e)", p=128
                    )
                    nc.sync.dma_start(out_view, emb_tile[:, :])
        if internal_out:
            with tc.tile_pool(name="d", bufs=1) as dp:
                dt_ = dp.tile([1, 16], mybir.dt.float32)
                nc.vector.memset(dt_[:, :], 0.0)
                nc.sync.dma_start(chk[:, :], dt_[:, :])
    nc.compile()
    return nc


def _marshal_idx(x_flat_core):
    """[N_PER_CORE] int -> [NCHUNK, 128, Q//16] int16 dma_gather operand.

    List position i of chunk k must hold the id of output vector
    k*Q + (i%128)*C + (i//128); positions are then 16-wrapped
    (wrapped[p, s] = pos[s*16+p]) and replicated to 128 partitions.
    """
    i = np.arange(Q)
    perm = (i % 128) * C + (i // 128)
    pos = x_flat_core.reshape(NCHUNK, Q)[:, perm]
    wrapped = pos.reshape(NCHUNK, Q // 16, 16).transpose(0, 2, 1)
    return np.ascontiguousarray(np.tile(wrapped, (1, 8, 1)).astype(np.int16))


def kernel(x, char2vec):
    from concourse.bass_utils import run_bass_kernel_spmd

    x = np.asarray(x)
    char2vec = np.ascontiguousarray(np.asarray(char2vec, dtype=np.float32))
    assert x.shape == (T, F, L), x.shape
    assert char2vec.shape == (VOCAB, EMB), char2vec.shape

    if "nc" not in _CACHE:
        _CACHE["nc"] = _build_nc()
    nc = _CACHE["nc"]

    x_shards = x.reshape(NCORES, N_PER_CORE)
    in_maps = [
        {"idx": _marshal_idx(x_shards[i]), "table": char2vec}
        for i in range(NCORES)
    ]
    res = run_bass_kernel_spmd(nc, in_maps, core_ids=list(range(NCORES)))
    out = np.concatenate([r["out"] for r in res.results], axis=0)
    return out.reshape(T, F, L * EMB)


# revision 7
# speedup vs baseline: 2.2781x; 2.2781x over previous
"""Embedding lookup (char-to-vector) on 8 Trainium2 NeuronCores.

Reference computation: out[t, f, l*64:(l+1)*64] = char2vec[x[t, f, l]]
with x: [256, 256, 8] int ids, char2vec: [8000, 64] f32.

Strategy (data-parallel, per the sharding hint):
  - Shard x along the first (timestep) dim: 32 rows -> 65536 lookups per core.
  - Replicate the 2 MB embedding table to every core (stays in HBM; the
    gather reads it directly).
  - On each core, loop over chunks of Q indices:
      idx DMA-in -> gpsimd.dma_gather (SWDGE gather, 256 B per index)
      -> HWDGE DMA-out of the gathered [128, Q/128, 64] tile.
  - Indices are pre-permuted host-side so that dma_gather's natural SBUF
    layout (index at list position i lands on partition i%128, column
    i//128) dumps to DRAM as the exact row-major [N, 64] output — no
    on-chip or host transpose.

dma_gather operand details (measured/validated on HW):
  - index operand: int16, SBUF [128, Q/16]; logical position i is read from
    partition i%16, column i//16, and the 16-partition block must be
    replicated 8x (one copy per Q7 GPSIMD core).
  - single_packet=False is required for Q > 1024: packet coalescing hits
    the 64-descriptor packet ceiling and wedges the SDMA engine.
  - per-queue throughput is descriptor-bound at ~8-9.5 ns per 256 B
    descriptor regardless of chunk size/packet mode/locality; rotating the
    gathers across all 4 SWDGE queues (num_swdge_queues=4, queue_num=k%4)
    overlaps ring generation/drain and runs ~2.4x faster end to end.
"""

import numpy as np

VOCAB = 8000
EMB = 64
T, F, L = 256, 256, 8
NCORES = 8
N_PER_CORE = (T // NCORES) * F * L  # 65536 lookups per core
Q = 4096                            # indices per dma_gather chunk
NQUEUES = 4                         # SWDGE queues; rotating queues ~2.4x gather rate
NCHUNK = N_PER_CORE // Q
C = Q // 128                        # vectors per partition per chunk

_CACHE = {}


def _build_nc(reps=1, internal_out=False):
    """Per-core program. reps>1 wraps the chunk loop in a hardware loop
    (used only for differential timing); internal_out keeps the big output
    in device DRAM (timing builds only)."""
    import concourse.bacc as bacc
    import concourse.mybir as mybir
    from concourse.tile import TileContext
    from concourse.library_config import mlp

    nc = bacc.Bacc(
        "TRN2", target_bir_lowering=False, debug=False,
        num_swdge_queues=NQUEUES,
    )
    idx = nc.dram_tensor(
        "idx", [NCHUNK, 128, Q // 16], mybir.dt.int16, kind="ExternalInput"
    )
    table = nc.dram_tensor(
        "table", [VOCAB, EMB], mybir.dt.float32, kind="ExternalInput"
    )
    out = nc.dram_tensor(
        "out",
        [N_PER_CORE, EMB],
        mybir.dt.float32,
        kind="Internal" if internal_out else "ExternalOutput",
    )
    chk = None
    if internal_out:
        chk = nc.dram_tensor("chk", [1, 16], mybir.dt.float32, kind="ExternalOutput")
    with TileContext(nc) as tc:
        nc.gpsimd.load_library(mlp)
        with (
            tc.tile_pool(name="idxp", bufs=4) as idxp,
            tc.tile_pool(name="embp", bufs=8) as embp,
        ):
            with tc.For_i(0, reps, 1):
                for k in range(NCHUNK):
                    idx_tile = idxp.tile([128, Q // 16], mybir.dt.int16)
                    nc.sync.dma_start(idx_tile[:, :], idx[k, :, :])
                    emb_tile = embp.tile([128, C * EMB], mybir.dt.float32)
                    emb3 = emb_tile[:, :].rearrange("p (c e) -> p c e", e=EMB)
                    nc.gpsimd.dma_gather(
                        emb3, table[:, :], idx_tile[:, :], Q, Q, EMB,
                        single_packet=False, queue_num=k % NQUEUES,
                    )
                    out_view = out[k * Q:(k + 1) * Q, :].rearrange(
                        "(p c) e -> p (c e)", p=128
                    )
                    nc.sync.dma_start(out_view, emb_tile[:, :])
        if internal_out:
            with tc.tile_pool(name="d", bufs=1) as dp:
                dt_ = dp.tile([1, 16], mybir.dt.float32)
                nc.vector.memset(dt_[:, :], 0.0)
                nc.sync.dma_start(chk[:, :], dt_[:, :])
    nc.compile()
    return nc


def _marshal_idx(x_flat_core):
    """[N_PER_CORE] int -> [NCHUNK, 128, Q//16] int16 dma_gather operand.

    List position i of chunk k must hold the id of output vector
    k*Q + (i%128)*C + (i//128); positions are then 16-wrapped
    (wrapped[p, s] = pos[s*16+p]) and replicated to 128 partitions.
    """
    i = np.arange(Q)
    perm = (i % 128) * C + (i // 128)
    pos = x_flat_core.reshape(NCHUNK, Q)[:, perm]
    wrapped = pos.reshape(NCHUNK, Q // 16, 16).transpose(0, 2, 1)
    return np.ascontiguousarray(np.tile(wrapped, (1, 8, 1)).astype(np.int16))


def kernel(x, char2vec):
    from concourse.bass_utils import run_bass_kernel_spmd

    x = np.asarray(x)
    char2vec = np.ascontiguousarray(np.asarray(char2vec, dtype=np.float32))
    assert x.shape == (T, F, L), x.shape
    assert char2vec.shape == (VOCAB, EMB), char2vec.shape

    if "nc" not in _CACHE:
        _CACHE["nc"] = _build_nc()
    nc = _CACHE["nc"]

    x_shards = x.reshape(NCORES, N_PER_CORE)
    in_maps = [
        {"idx": _marshal_idx(x_shards[i]), "table": char2vec}
        for i in range(NCORES)
    ]
    res = run_bass_kernel_spmd(nc, in_maps, core_ids=list(range(NCORES)))
    out = np.concatenate([r["out"] for r in res.results], axis=0)
    return out.reshape(T, F, L * EMB)
